# revision 11
# baseline (speedup 1.0000x reference)
"""Causal single-head attention (B=2, S=4096, D=1024) with RoPE on 8 TRN2 NeuronCores.

Sharding: per batch element, the 32 kv chunks (128 rows) are dealt round-robin
to 4 cores (chunk k -> core k%4). Every core runs an identical 32-slot program:
slot j computes partial causal attention of query chunk j (128 rows) against
the first sched[j] = 128*(j//4+1) rows of the core's gathered kv buffer, with
host-provided additive causal masks (which also kill not-owned columns).
Cores return unnormalized partials (o_un, rowmax, rowsum); the host merges the
4 partial softmaxes per query row and normalizes.

All matmuls run in bf16 with fp32 PSUM accumulation. Q/K output features are
permuted (evens-then-odds) on the host so RoPE operates on contiguous halves;
the permutation cancels in Q.K^T.
"""

import os
import sys

sys.path.insert(0, "/opt/trn_rl_repo")

import math
from contextlib import ExitStack

import ml_dtypes
import numpy as np

import concourse.bass as bass
import concourse.tile as tile
from concourse import bacc, mybir
from concourse.bass_utils import run_bass_kernel_spmd
from concourse.masks import make_identity

BF16 = mybir.dt.bfloat16
F32 = mybir.dt.float32
NPBF16 = ml_dtypes.bfloat16

B, S, D = 2, 4096, 1024
H = D // 2
C = 128                      # chunk rows
NQC = S // C                 # 32 query-chunk slots
NKVC = NQC // 4              # 8 kv chunks per core
NKV = NKVC * C               # 1024 resident kv rows per core
SCHED = [C * (j // 4 + 1) for j in range(NQC)]   # static kv window per slot
MOFF = [sum(SCHED[:j]) for j in range(NQC)]      # mask column offsets
MTOT = sum(SCHED)
QG = 256                     # phase-B query group rows (2 slots)
NG = S // QG                 # 16 groups
SCALE = 1.0 / math.sqrt(D)
NEG = -30000.0

_CACHE = {}
KPHASE = os.environ.get("KPHASE", "all")


def _build():
    """Build + schedule the (core-uniform) Bass program once."""
    nc = bacc.Bacc("TRN2", target_bir_lowering=False, debug=False,
                   enable_asserts=False, num_devices=8)

    x_q = nc.dram_tensor("x_q", [S, D], BF16, kind="ExternalInput").ap()
    x_kv = nc.dram_tensor("x_kv", [NKV, D], BF16, kind="ExternalInput").ap()
    wqT = nc.dram_tensor("wqT", [D, D], BF16, kind="ExternalInput").ap()
    wkT = nc.dram_tensor("wkT", [D, D], BF16, kind="ExternalInput").ap()
    wvT = nc.dram_tensor("wvT", [D, D], BF16, kind="ExternalInput").ap()
    cosT = nc.dram_tensor("cosT", [H, S], BF16, kind="ExternalInput").ap()
    sinT = nc.dram_tensor("sinT", [H, S], BF16, kind="ExternalInput").ap()
    cos_kv = nc.dram_tensor("cos_kv", [NKV, H], BF16, kind="ExternalInput").ap()
    sin_kv = nc.dram_tensor("sin_kv", [NKV, H], BF16, kind="ExternalInput").ap()
    masks = nc.dram_tensor("masks", [C, MTOT], F32, kind="ExternalInput").ap()

    o_un = nc.dram_tensor("o_un", [NQC, C, D], F32, kind="ExternalOutput").ap()
    stats = nc.dram_tensor("stats", [C, NQC, 2], F32, kind="ExternalOutput").ap()

    with tile.TileContext(nc) as tc, ExitStack() as ctx:
        # ---------------- resident pools ----------------
        const_p = ctx.enter_context(tc.tile_pool(name="const", bufs=1))
        w_p = ctx.enter_context(tc.tile_pool(name="weights", bufs=1))
        kvres_p = ctx.enter_context(tc.tile_pool(name="kvres", bufs=1))
        stats_p = ctx.enter_context(tc.tile_pool(name="stats", bufs=1))

        ident = const_p.tile([C, C], BF16)
        make_identity(nc, ident[:])

        wq_sb = w_p.tile([C, 8, D], BF16, tag="wq")
        wk_sb = w_p.tile([C, 8, D], BF16, tag="wk")
        wv_sb = w_p.tile([C, 8, D], BF16, tag="wv")
        nc.sync.dma_start(wq_sb[:], wqT.rearrange("(dc p) e -> p dc e", p=C))
        nc.sync.dma_start(wk_sb[:], wkT.rearrange("(dc p) e -> p dc e", p=C))
        nc.sync.dma_start(wv_sb[:], wvT.rearrange("(dc p) e -> p dc e", p=C))

        kt_sb = kvres_p.tile([C, 8, NKV], BF16, tag="kt")     # [p, dc, kvpos]
        v_sb = kvres_p.tile([C, NKVC, D], BF16, tag="v")      # [p, kvchunk, d]
        stats_sb = stats_p.tile([C, NQC, 2], F32, tag="st")

        # ---------------- phase A1: K,V of resident kv rows ----------------
        with tc.tile_pool(name="a1", bufs=2) as a1_p, \
             tc.tile_pool(name="a1ps", bufs=2, space="PSUM") as a1ps_p, \
             tc.tile_pool(name="a1kv", bufs=1, space="PSUM") as a1kv_p:
            for g in range(NKVC):
                rows = slice(g * C, (g + 1) * C)
                xkv_sb = a1_p.tile([C, D], BF16, tag="xkv")
                nc.sync.dma_start(xkv_sb[:], x_kv[rows, :])
                ckv_sb = a1_p.tile([C, H], BF16, tag="ckv")
                skv_sb = a1_p.tile([C, H], BF16, tag="skv")
                nc.sync.dma_start(ckv_sb[:], cos_kv[rows, :])
                nc.sync.dma_start(skv_sb[:], sin_kv[rows, :])

                xt_sb = a1_p.tile([C, 8, C], BF16, tag="xtkv")
                for dc in range(8):
                    tp = a1ps_p.tile([C, 1024], BF16, tag="xtp")
                    nc.tensor.transpose(tp[:, 0:C], xkv_sb[:, dc * C:(dc + 1) * C], ident[:])
                    nc.scalar.copy(xt_sb[:, dc, :], tp[:, 0:C])

                k_ps = a1kv_p.tile([C, D], F32, tag="kps")
                v_ps = a1kv_p.tile([C, D], F32, tag="vps")
                for h in range(2):
                    cols = slice(h * 512, (h + 1) * 512)
                    for dc in range(8):
                        nc.tensor.matmul(k_ps[:, cols], xt_sb[:, dc, :],
                                         wk_sb[:, dc, cols],
                                         start=(dc == 0), stop=(dc == 7))
                    for dc in range(8):
                        nc.tensor.matmul(v_ps[:, cols], xt_sb[:, dc, :],
                                         wv_sb[:, dc, cols],
                                         start=(dc == 0), stop=(dc == 7))
                nc.scalar.copy(v_sb[:, g, :], v_ps[:])

                # rope K in natural layout (halves are real|imag after permutation)
                kr_sb = a1_p.tile([C, D], BF16, tag="kr")
                t0 = a1_p.tile([C, H], BF16, tag="t0")
                t1 = a1_p.tile([C, H], BF16, tag="t1")
                re, im = k_ps[:, 0:H], k_ps[:, H:D]
                nc.vector.tensor_mul(t0[:], re, ckv_sb[:])
                nc.vector.tensor_mul(t1[:], im, skv_sb[:])
                nc.vector.tensor_sub(kr_sb[:, 0:H], t0[:], t1[:])
                nc.vector.tensor_mul(t0[:], re, skv_sb[:])
                nc.vector.tensor_mul(t1[:], im, ckv_sb[:])
                nc.vector.tensor_add(kr_sb[:, H:D], t0[:], t1[:])

                for dc in range(8):
                    tp = a1ps_p.tile([C, 1024], BF16, tag="ktp")
                    nc.tensor.transpose(tp[:, 0:C], kr_sb[:, dc * C:(dc + 1) * C], ident[:])
                    nc.scalar.copy(kt_sb[:, dc, g * C:(g + 1) * C], tp[:, 0:C])

        # -------- phase A2+B fused: project+rope Q group, then attend --------
        with tc.tile_pool(name="b", bufs=2) as b_p, \
             tc.tile_pool(name="bq", bufs=2) as bq_p, \
             tc.tile_pool(name="bs", bufs=2) as bs_p, \
             tc.tile_pool(name="qtps", bufs=2, space="PSUM") as qtps_p, \
             tc.tile_pool(name="sps", bufs=2, space="PSUM") as sps_p, \
             tc.tile_pool(name="ptps", bufs=2, space="PSUM") as ptps_p, \
             tc.tile_pool(name="ops", bufs=1, space="PSUM") as ops_p:
            for g in range(NG if KPHASE != "a1" else 0):
                rows = slice(g * QG, (g + 1) * QG)
                xg_sb = b_p.tile([C, 2, D], BF16, tag="xg")
                nc.sync.dma_start(xg_sb[:], x_q[rows, :].rearrange("(a p) d -> p a d", p=C))
                ct_sb = b_p.tile([C, 4, QG], BF16, tag="ct")
                st_sb = b_p.tile([C, 4, QG], BF16, tag="st")
                nc.sync.dma_start(ct_sb[:], cosT[:, rows].rearrange("(c p) s -> p c s", p=C))
                nc.sync.dma_start(st_sb[:], sinT[:, rows].rearrange("(c p) s -> p c s", p=C))

                xt_sb = b_p.tile([C, 8, QG], BF16, tag="xtq")
                for dc in range(8):
                    tp = qtps_p.tile([C, 1024], BF16, tag="qtp")
                    for a in range(2):
                        nc.tensor.transpose(tp[:, a * C:(a + 1) * C],
                                            xg_sb[:, a, dc * C:(dc + 1) * C], ident[:])
                    nc.scalar.copy(xt_sb[:, dc, :], tp[:, 0:QG])

                qraw_sb = bq_p.tile([C, 8, QG], BF16, tag="qraw")
                for ec in range(8):
                    qp = qtps_p.tile([C, 512], F32, tag="qtp")
                    for dc in range(8):
                        nc.tensor.matmul(qp[:, 0:QG], wq_sb[:, dc, ec * C:(ec + 1) * C],
                                         xt_sb[:, dc, :],
                                         start=(dc == 0), stop=(dc == 7))
                    nc.scalar.copy(qraw_sb[:, ec, :], qp[:, 0:QG])

                qt_sb = bq_p.tile([C, 8, QG], BF16, tag="qt")
                for ec in range(4):
                    cc, ss = ct_sb[:, ec, :], st_sb[:, ec, :]
                    re, im = qraw_sb[:, ec, :], qraw_sb[:, ec + 4, :]
                    t0 = b_p.tile([C, QG], BF16, tag="rt0")
                    t1 = b_p.tile([C, QG], BF16, tag="rt1")
                    nc.vector.tensor_mul(t0[:], re, cc)
                    nc.vector.tensor_mul(t1[:], im, ss)
                    nc.vector.tensor_sub(qt_sb[:, ec, :], t0[:], t1[:])
                    t2 = b_p.tile([C, QG], BF16, tag="rt2")
                    t3 = b_p.tile([C, QG], BF16, tag="rt3")
                    nc.vector.tensor_mul(t2[:], re, ss)
                    nc.vector.tensor_mul(t3[:], im, cc)
                    nc.vector.tensor_add(qt_sb[:, ec + 4, :], t2[:], t3[:])

                for jj in range(0 if KPHASE in ("a1", "qt") else 2):
                    j = 2 * g + jj
                    W = SCHED[j]
                    qc = slice(jj * C, (jj + 1) * C)

                    m_sb = bs_p.tile([C, 1024], F32, tag="mask")
                    nc.sync.dma_start(m_sb[:, 0:W], masks[:, MOFF[j]:MOFF[j] + W])
                    sc_sb = bs_p.tile([C, 1024], F32, tag="scores")
                    rmax = [bs_p.tile([C, 1], F32, tag=f"rmax{t}", name=f"rmax{t}_{j}")
                            for t in range(2)]

                    ntile = (W + 511) // 512
                    for t in range(ntile):
                        wt = min(512, W - t * 512)
                        cols = slice(t * 512, t * 512 + wt)
                        s_ps = sps_p.tile([C, 512], F32, tag="sps")
                        for dc in range(8):
                            nc.tensor.matmul(s_ps[:, 0:wt], qt_sb[:, dc, qc],
                                             kt_sb[:, dc, cols],
                                             start=(dc == 0), stop=(dc == 7))
                        nc.vector.tensor_add(sc_sb[:, cols], s_ps[:, 0:wt], m_sb[:, cols])

                    if KPHASE in ("s", "s2", "s3"):
                        continue
                    nc.vector.tensor_reduce(rmax[0][:], sc_sb[:, 0:W],
                                            axis=mybir.AxisListType.X,
                                            op=mybir.AluOpType.max)
                    negm = bs_p.tile([C, 1], F32, tag="negm")
                    nc.scalar.mul(negm[:], rmax[0][:], -SCALE)
                    p_sb = bs_p.tile([C, 1024], BF16, tag="p")
                    lsum = bs_p.tile([C, 1], F32, tag="lsum")
                    nc.scalar.activation(p_sb[:, 0:W], sc_sb[:, 0:W],
                                         mybir.ActivationFunctionType.Exp,
                                         bias=negm[:], scale=SCALE,
                                         accum_out=lsum[:])
                    nc.scalar.copy(stats_sb[:, j, 0:1], negm[:])
                    nc.scalar.copy(stats_sb[:, j, 1:2], lsum[:])

                    if KPHASE == "exp":
                        continue
                    o_ps = ops_p.tile([C, D], F32, tag="ops")
                    nsub = W // C
                    for sI in range(nsub):
                        ptp = ptps_p.tile([C, 1024], BF16, tag="ptp")
                        nc.tensor.transpose(ptp[:, 0:C], p_sb[:, sI * C:(sI + 1) * C], ident[:])
                        pt_sb = b_p.tile([C, C], BF16, tag="pt")
                        nc.scalar.copy(pt_sb[:], ptp[:, 0:C])
                        for h in range(2):
                            cols = slice(h * 512, (h + 1) * 512)
                            nc.tensor.matmul(o_ps[:, cols], pt_sb[:],
                                             v_sb[:, sI, cols],
                                             start=(sI == 0), stop=(sI == nsub - 1))
                    ob_sb = bs_p.tile([C, D], F32, tag="ob")
                    nc.scalar.copy(ob_sb[:], o_ps[:])
                    nc.sync.dma_start(o_un[j], ob_sb[:])

        if KPHASE in ("exp", "pv", "all"):
            nc.sync.dma_start(stats, stats_sb[:])

    nc.compile()
    return nc


def _prep_inputs(x, w_q, w_k, w_v, freqs_cos, freqs_sin):
    """Host-side per-core input maps (numpy)."""
    perm = np.concatenate([np.arange(0, D, 2), np.arange(1, D, 2)])
    wqT = np.ascontiguousarray(w_q[perm, :].T.astype(NPBF16))
    wkT = np.ascontiguousarray(w_k[perm, :].T.astype(NPBF16))
    wvT = np.ascontiguousarray(w_v.T.astype(NPBF16))
    cosT = np.ascontiguousarray(freqs_cos.T.astype(NPBF16))   # [H, S]
    sinT = np.ascontiguousarray(freqs_sin.T.astype(NPBF16))

    in_maps = []
    for core in range(8):
        b, i = divmod(core, 4)
        kcs = np.arange(i, NQC, 4)                            # owned kv chunks
        kvrows = (kcs[:, None] * C + np.arange(C)[None, :]).reshape(-1)  # [NKV]
        xb = np.asarray(x[b]).astype(NPBF16)
        m = np.full((C, MTOT), 0.0, np.float32)
        for j in range(NQC):
            W = SCHED[j]
            qg = j * C + np.arange(C)                          # global q rows
            kg = kvrows[:W]                                    # global k rows
            blk = np.where(kg[None, :] <= qg[:, None], 0.0, NEG)
            m[:, MOFF[j]:MOFF[j] + W] = blk
        in_maps.append({
            "x_q": xb,
            "x_kv": np.ascontiguousarray(xb[kvrows]),
            "wqT": wqT, "wkT": wkT, "wvT": wvT,
            "cosT": cosT, "sinT": sinT,
            "cos_kv": np.ascontiguousarray(freqs_cos[kvrows].astype(NPBF16)),
            "sin_kv": np.ascontiguousarray(freqs_sin[kvrows].astype(NPBF16)),
            "masks": m,
        })
    return in_maps


def _merge(results):
    """Host softmax-merge of per-core partials -> [B,S,D] f32."""
    out = np.zeros((B, S, D), np.float64)
    for b in range(B):
        for j in range(NQC):
            parts = []
            for i in range(min(j + 1, 4)):
                r = results[4 * b + i]
                mrow = -r["stats"][:, j, 0].astype(np.float64)       # [C]
                lrow = r["stats"][:, j, 1].astype(np.float64)
                orow = r["o_un"][j].astype(np.float64)               # [C, D]
                parts.append((mrow, lrow, orow))
            M = np.max(np.stack([p[0] for p in parts]), axis=0)
            num = np.zeros((C, D), np.float64)
            den = np.zeros((C,), np.float64)
            for mrow, lrow, orow in parts:
                w = np.exp(mrow - M)
                num += w[:, None] * orow
                den += w * lrow
            out[b, j * C:(j + 1) * C] = num / den[:, None]
    return out.astype(np.float32)


def kernel(x, w_q, w_k, w_v, freqs_cos, freqs_sin, _want_results=False, _trace=False):
    if "nc" not in _CACHE:
        _CACHE["nc"] = _build()
    nc = _CACHE["nc"]
    in_maps = _prep_inputs(np.asarray(x, np.float32), np.asarray(w_q, np.float32),
                           np.asarray(w_k, np.float32), np.asarray(w_v, np.float32),
                           np.asarray(freqs_cos, np.float32),
                           np.asarray(freqs_sin, np.float32))
    kr = run_bass_kernel_spmd(nc, in_maps, core_ids=list(range(8)), trace=_trace)
    out = _merge(kr.results)
    if _want_results:
        return out, kr
    return out
